# revision 8
# baseline (speedup 1.0000x reference)
"""Trainium2 Bass kernel for nn_BasicGNN (4-layer GCN + 2 interleaved convs,
layer norms, time-embedding MLP) on N=10000 nodes / E=160000 edges.

Strategy: graph-parallel over 8 NeuronCores. Nodes are sharded contiguously
(1250/core). Per conv layer, each core:
  1. computes its h = x @ W shard on the PE (float32r),
  2. writes h (bf16, rows padded to 768) to HBM and AllGathers it,
  3. gathers the rows its local edges need with dma_gather and
     scatter-reduces them into dst rows via "staircase" matmuls whose
     stationary operand S carries the GCN edge coefficients (self loops and
     the conv bias ride along as extra slots),
  4. applies relu / layernorm / time-MLP epilogue on ACT+DVE.
Graph structure (edge partition, S matrices, gather indices) is precomputed
on the host from the actual edge_index; it is identical for all 6
message-passing rounds.
"""
import numpy as np
import ml_dtypes

import concourse.bass as bass
import concourse.tile as tile
from concourse import bacc, mybir
from concourse.bass_utils import run_bass_kernel_spmd

bf16 = ml_dtypes.bfloat16

# Problem shapes (hardcoded per spec nn_BasicGNN_41248865911519)
N, E = 10000, 160000
IN_C, HID, OUT_C, TDIM = 178, 712, 178, 32
EPS = 1e-5

NCORES = 8
NP = N // NCORES            # 1250 nodes per core
NT = 10                     # node tiles per core (9 full + 98)
NPAD = NT * 128             # 1280
HPAD = 768                  # padded hidden row (bf16 rows = 1536B, /256 ok)
INPAD = 256                 # padded input features (2 k-tiles)
OPAD = 256                  # padded output features (bf16 rows = 512B)
NBLK = 10                   # dst blocks of 128 per core
PIECE = 6                   # gather chunks per dma_gather call
BIAS_ROW = 1250             # local row in h_my carrying this layer's bias

F32, F32R, BF16, I16 = (mybir.dt.float32, mybir.dt.float32r,
                        mybir.dt.bfloat16, mybir.dt.int16)
AF = mybir.ActivationFunctionType
ALU = mybir.AluOpType


# ---------------------------------------------------------------- host prep
def _preprocess(edge_index):
    """Partition edges by dst core/block, dedup (block, src) into slots,
    build S matrices + gather index lists. Slot layout per (core, block):
    [unique srcs..., bias slot, padding]."""
    src = np.asarray(edge_index[0], np.int64)
    dst = np.asarray(edge_index[1], np.int64)
    deg = np.bincount(dst, minlength=N).astype(np.float32) + 1.0
    dinv = 1.0 / np.sqrt(deg)
    # append self loops
    a_src = np.concatenate([src, np.arange(N)])
    a_dst = np.concatenate([dst, np.arange(N)])
    a_coef = np.concatenate([dinv[src] * dinv[dst], dinv * dinv]).astype(np.float32)

    core = a_dst // NP
    loc = a_dst % NP
    blk = loc // 128
    row = loc % 128

    slots = [[None] * NBLK for _ in range(NCORES)]
    smats = [[None] * NBLK for _ in range(NCORES)]
    order = np.lexsort((a_src, blk, core))
    csrc, cblk, ccore = a_src[order], blk[order], core[order]
    crow, ccoef = row[order], a_coef[order]
    bounds = np.searchsorted(ccore * NBLK + cblk, np.arange(NCORES * NBLK + 1))
    max_nslot = 0
    for c in range(NCORES):
        for b in range(NBLK):
            g = slice(bounds[c * NBLK + b], bounds[c * NBLK + b + 1])
            gs, gr, gc = csrc[g], crow[g], ccoef[g]
            uniq, inv = np.unique(gs, return_inverse=True)
            S = np.zeros((len(uniq), 128), np.float32)
            np.add.at(S, (inv, gr), gc)
            slots[c][b] = uniq
            smats[c][b] = S
            max_nslot = max(max_nslot, len(uniq))
    nchunk = (max_nslot + 1 + 127) // 128   # +1 for the bias slot
    tot = NBLK * nchunk * 128

    def gid(n):  # padded global row id in the AllGathered buffer
        return (n // NP) * NPAD + (n % NP)

    gidx = np.zeros((NCORES, tot), np.int16)
    smat = np.zeros((NCORES, 128, NBLK * nchunk, 128), bf16)
    for c in range(NCORES):
        for b in range(NBLK):
            u = slots[c][b]
            S = smats[c][b]
            ns = len(u)
            base = b * nchunk * 128
            gidx[c, base:base + ns] = gid(u)
            for ch in range((ns + 127) // 128):
                lo, hi = ch * 128, min((ch + 1) * 128, ns)
                smat[c, 0:hi - lo, b * nchunk + ch, :] = S[lo:hi].astype(bf16)
            # bias slot: reads this core's h_my[BIAS_ROW], coef 1 everywhere
            gidx[c, base + ns] = c * NPAD + BIAS_ROW
            smat[c, ns % 128, b * nchunk + ns // 128, :] = bf16(1.0)
    # wrap: idx i at [i % 16, i // 16], replicated across the 8 Q7 cores
    gidx_w = gidx.reshape(NCORES, tot // 16, 16).transpose(0, 2, 1)
    gidx_w = np.tile(gidx_w, (1, 8, 1)).copy()
    return nchunk, gidx_w, smat


def _host_tmlp(timestep, time_w1, time_b1, time_ws, time_bs):
    t = np.asarray(timestep).astype(np.float32).reshape(-1)
    half = TDIM // 2
    freqs = np.exp(np.arange(half, dtype=np.float32)
                   * (-np.log(10000.0) / (half - 1)))
    args = t[:, None] * freqs[None, :]
    emb = np.concatenate([np.sin(args), np.cos(args)], -1).astype(np.float32)
    tm = np.maximum(emb @ np.asarray(time_w1, np.float32)
                    + np.asarray(time_b1, np.float32), 0.0)
    return [np.asarray(tm @ np.asarray(time_ws[j], np.float32)
                       + np.asarray(time_bs[j], np.float32)).reshape(-1)
            for j in range(2)]


def _padw(w, rpad, cpad):
    w = np.asarray(w, np.float32)
    out = np.zeros((rpad, cpad), np.float32)
    out[:w.shape[0], :w.shape[1]] = w
    return out


def _rep(v, cpad):
    v = np.asarray(v, np.float32).reshape(-1)
    out = np.zeros((128, cpad), np.float32)
    out[:, :v.shape[0]] = v[None, :]
    return out


# per-layer dense configs: (ktiles, outpad, out_valid)
LCFG = [(2, HPAD, HID), (6, HPAD, HID), (6, HPAD, HID),
        (6, HPAD, HID), (6, HPAD, HID), (6, OPAD, OUT_C)]
# post-ops per layer: (relu, ln param index or None)
POST = [(True, None), (True, 0), (True, 1), (True, 2), (True, 3),
        (False, None)]


# ------------------------------------------------------------- bass program
def _build(nchunk):
    tot = NBLK * nchunk * 128
    nc = bacc.Bacc("TRN2", target_bir_lowering=False, debug=False,
                   num_devices=NCORES)

    xT_in = nc.dram_tensor("xT", [128, 2, NPAD], F32R, kind="ExternalInput")
    w_in = [nc.dram_tensor(f"w{i}", [128, kt, op], F32R, kind="ExternalInput")
            for i, (kt, op, _) in enumerate(LCFG)]
    biasrow_in = nc.dram_tensor("biasrow", [1, 6, HPAD], BF16,
                                kind="ExternalInput")
    lng_in = [nc.dram_tensor(f"lng{i}", [128, HPAD], F32, kind="ExternalInput")
              for i in range(4)]
    lnb_in = [nc.dram_tensor(f"lnb{i}", [128, HPAD], F32, kind="ExternalInput")
              for i in range(4)]
    s_in = nc.dram_tensor("smat", [128, NBLK * nchunk, 128], BF16,
                          kind="ExternalInput")
    gidx_in = nc.dram_tensor("gidx", [128, tot // 16], I16,
                             kind="ExternalInput")

    out_dram = nc.dram_tensor("out", [NT, 128, OPAD], F32,
                              kind="ExternalOutput")

    h_my = nc.dram_tensor("h_my", [NPAD, HPAD], BF16)
    h_full = nc.dram_tensor("h_full", [NCORES * NPAD, HPAD], BF16,
                            addr_space="Shared")
    h_my_sm = nc.dram_tensor("h_my_sm", [NPAD, OPAD], BF16)
    h_full_sm = nc.dram_tensor("h_full_sm", [NCORES * NPAD, OPAD], BF16,
                               addr_space="Shared")

    with tile.TileContext(nc) as tc:
        import contextlib
        with contextlib.ExitStack() as ctx:
            _body(ctx, tc, nc, nchunk, xT_in, w_in, biasrow_in, lng_in,
                  lnb_in, s_in, gidx_in, out_dram, h_my, h_full,
                  h_my_sm, h_full_sm)
    nc.compile()
    return nc


def _body(ctx, tc, nc, nchunk, xT_in, w_in, biasrow_in, lng_in, lnb_in,
          s_in, gidx_in, out_dram, h_my, h_full, h_my_sm, h_full_sm):
    tot = NBLK * nchunk * 128

    const = ctx.enter_context(tc.tile_pool(name="const", bufs=1))
    wpool = ctx.enter_context(tc.tile_pool(name="wpool", bufs=1))
    act_pool = ctx.enter_context(tc.tile_pool(name="act", bufs=1))
    xt_pool = ctx.enter_context(tc.tile_pool(name="xt", bufs=4))
    hbf_pool = ctx.enter_context(tc.tile_pool(name="hbf", bufs=3))
    gath_pool = ctx.enter_context(tc.tile_pool(name="gath", bufs=3))
    ep_pool = ctx.enter_context(tc.tile_pool(name="ep", bufs=2))
    st_pool = ctx.enter_context(tc.tile_pool(name="st", bufs=4))
    psum_h = ctx.enter_context(tc.tile_pool(name="ph", bufs=1, space="PSUM"))
    psum_t = ctx.enter_context(tc.tile_pool(name="pt", bufs=2, space="PSUM"))
    psum_a = ctx.enter_context(tc.tile_pool(name="pa", bufs=2, space="PSUM"))

    # ---- persistent loads
    smat = const.tile([128, NBLK * nchunk, 128], BF16)
    nc.sync.dma_start(smat[:], s_in[:, :, :])
    gidx = const.tile([128, tot // 16], I16)
    nc.sync.dma_start(gidx[:], gidx_in[:, :])
    biasrow = const.tile([1, 6, HPAD], BF16)
    nc.sync.dma_start(biasrow[:], biasrow_in[:, :, :])
    lngs, lnbs = [], []
    for i in range(4):
        g = const.tile([128, HPAD], F32, tag=f"lng{i}")
        nc.sync.dma_start(g[:], lng_in[i][:, :])
        lngs.append(g)
        b = const.tile([128, HPAD], F32, tag=f"lnb{i}")
        nc.sync.dma_start(b[:], lnb_in[i][:, :])
        lnbs.append(b)
    ident = const.tile([128, 128], F32)
    nc.sync.dma_start(ident[:], nc.inline_tensor(np.eye(128, dtype=np.float32),
                                                 "ident").ap())
    xT0 = const.tile([128, 2, NPAD], F32R)
    nc.sync.dma_start(xT0[:], xT_in[:, :, :])

    # activation (node-major), rewritten by each layer's epilogue
    act = act_pool.tile([128, NT, HPAD], F32)

    for L, (kt_n, op, ov) in enumerate(LCFG):
        hm = h_my if L < 5 else h_my_sm
        hf = h_full if L < 5 else h_full_sm
        # ---------- phase A: dense h = x @ W -> bf16 -> HBM
        w_r = wpool.tile([128, kt_n, op], F32R, tag="w_r")
        nc.sync.dma_start(w_r[:], w_in[L][:, :, :])
        for nt in range(NT):
            hp = psum_h.tile([128, op], F32, tag="hpsum")
            for kt in range(kt_n):
                if L == 0:
                    xt = xT0[:, kt, nt * 128:(nt + 1) * 128]
                else:
                    tp = psum_t.tile([128, 128], F32, tag="tp")
                    nc.tensor.transpose(
                        tp[:], act[:, nt, kt * 128:(kt + 1) * 128], ident[:])
                    xts = xt_pool.tile([128, 128], F32R, tag="xt")
                    nc.scalar.copy(xts[:], tp[:])
                    xt = xts[:]
                for lo in range(0, op, 512):
                    hi = min(lo + 512, op)
                    nc.tensor.matmul(hp[:, lo:hi], xt, w_r[:, kt, lo:hi],
                                     start=(kt == 0), stop=(kt == kt_n - 1))
            hbf = hbf_pool.tile([128, op], BF16, tag="hbf")
            nc.vector.tensor_copy(hbf[:], hp[:])
            nc.sync.dma_start(hm[nt * 128:(nt + 1) * 128, :], hbf[:])
        # bias row for the aggregation's bias slot
        nc.sync.dma_start(hm[BIAS_ROW:BIAS_ROW + 1, :op],
                          biasrow[0:1, L, 0:op])
        # ---------- phase B: allgather
        nc.gpsimd.collective_compute(
            "AllGather", ALU.bypass,
            replica_groups=[list(range(NCORES))],
            ins=[hm.ap().opt()], outs=[hf.ap().opt()])
        # ---------- phase C: gather + aggregate + epilogue per dst block
        relu, ln = POST[L]
        for b in range(NBLK):
            ap_ = psum_a.tile([128, op], F32, tag="apsum")
            npieces = (nchunk + PIECE - 1) // PIECE
            for p in range(npieces):
                c0, c1 = p * PIECE, min((p + 1) * PIECE, nchunk)
                g = gath_pool.tile([128, c1 - c0, op], BF16, tag="gath")
                i0 = (b * nchunk + c0) * 128
                i1 = (b * nchunk + c1) * 128
                nc.gpsimd.dma_gather(
                    g[:], hf.ap(), gidx[:, i0 // 16:i1 // 16],
                    i1 - i0, i1 - i0, op)
                for c in range(c0, c1):
                    for lo in range(0, op, 512):
                        hi = min(lo + 512, op)
                        nc.tensor.matmul(
                            ap_[:, lo:hi], smat[:, b * nchunk + c, :],
                            g[:, c - c0, lo:hi],
                            start=(c == 0), stop=(c == nchunk - 1))
            _epilogue(nc, ep_pool, st_pool, ap_, act, out_dram, b, relu, ln,
                      lngs, lnbs, op, ov, L)


def _epilogue(nc, ep_pool, st_pool, ap_, act, out_dram, b, relu, ln,
              lngs, lnbs, op, ov, L):
    """psum [128, op] (already includes bias) -> relu -> LN*g+b -> dst"""
    if L == 5:
        t = ep_pool.tile([128, op], F32, tag="ep5")
        nc.vector.tensor_copy(t[:], ap_[:])
        nc.sync.dma_start(out_dram[b, :, :], t[:])
        return
    dstv = act[:, b, :]
    if ln is None:
        nc.scalar.activation(dstv, ap_[:], AF.Relu)
        return
    # relu (psum -> sbuf) with fused sum; then square with fused sum
    t = ep_pool.tile([128, op], F32, tag="ep")
    stats = st_pool.tile([128, 2], F32, tag="st")
    nc.scalar.activation(t[:], ap_[:], AF.Relu, accum_out=stats[:, 0:1])
    x = t[:, 0:ov]
    sq = ep_pool.tile([128, ov], F32, tag="scr")
    nc.scalar.activation(sq[:], x, AF.Square, accum_out=stats[:, 1:2])
    nmu = st_pool.tile([128, 1], F32, tag="nmu")
    nc.scalar.mul(nmu[:], stats[:, 0:1], -1.0 / ov)
    mu2 = st_pool.tile([128, 1], F32, tag="mu2")
    nc.vector.tensor_mul(mu2[:], nmu[:], nmu[:])
    var = st_pool.tile([128, 1], F32, tag="var")
    nc.vector.tensor_scalar(var[:], stats[:, 1:2], 1.0 / ov, None,
                            op0=ALU.mult)
    nc.vector.tensor_sub(var[:], var[:], mu2[:])
    nc.vector.tensor_scalar_add(var[:], var[:], EPS)
    rvar = st_pool.tile([128, 1], F32, tag="rvar")
    nc.vector.reciprocal(rvar[:], var[:])
    rstd = st_pool.tile([128, 1], F32, tag="rstd")
    nc.scalar.activation(rstd[:], rvar[:], AF.Sqrt)
    y = ep_pool.tile([128, ov], F32, tag="scr")
    nc.vector.tensor_scalar(y[:], x, nmu[:], rstd[:],
                            op0=ALU.add, op1=ALU.mult)
    g, bb = lngs[ln], lnbs[ln]
    nc.vector.tensor_mul(y[:], y[:], g[:, 0:ov])
    nc.vector.tensor_add(act[:, b, 0:ov], y[:], bb[:, 0:ov])


# ------------------------------------------------------------------- driver
_CACHE = {}


def _prepare(inputs):
    x = np.asarray(inputs["x"], np.float32)
    edge_index = np.asarray(inputs["edge_index"])
    nchunk, gidx_w, smat = _preprocess(edge_index)

    tadds = _host_tmlp(inputs["timestep"], inputs["time_w1"],
                       inputs["time_b1"], inputs["time_ws"], inputs["time_bs"])

    conv_ws = [np.asarray(w, np.float32) for w in inputs["conv_ws"]]
    conv_bs = [np.asarray(w, np.float32) for w in inputs["conv_bs"]]
    conv2_ws = [np.asarray(w, np.float32) for w in inputs["conv2_ws"]]
    conv2_bs = [np.asarray(w, np.float32) for w in inputs["conv2_bs"]]
    ln1_gs = [np.asarray(w, np.float32) for w in inputs["ln1_gs"]]
    ln1_bs = [np.asarray(w, np.float32) for w in inputs["ln1_bs"]]
    ln2_gs = [np.asarray(w, np.float32) for w in inputs["ln2_gs"]]
    ln2_bs = [np.asarray(w, np.float32) for w in inputs["ln2_bs"]]

    # layer order: conv0, conv1, conv2_0, conv2, conv2_1, conv3
    Ws = [conv_ws[0], conv_ws[1], conv2_ws[0], conv_ws[2], conv2_ws[1],
          conv_ws[3]]
    Bs = [conv_bs[0], conv_bs[1], conv2_bs[0], conv_bs[2], conv2_bs[1],
          conv_bs[3]]
    w_maps = []
    biasrow = np.zeros((1, 6, HPAD), bf16)
    for i, (kt, op, _) in enumerate(LCFG):
        rp = INPAD if i == 0 else HPAD
        wp = _padw(Ws[i], rp, op)
        w_maps.append(wp.reshape(kt, 128, op).transpose(1, 0, 2).copy())
        biasrow[0, i, :op] = _padw(Bs[i].reshape(1, -1), 1, op)[0].astype(bf16)
    lng = [_rep(ln1_gs[0], HPAD), _rep(ln2_gs[0], HPAD),
           _rep(ln1_gs[1], HPAD), _rep(ln2_gs[1], HPAD)]
    lnb = [_rep(np.asarray(ln1_bs[0]) + tadds[0], HPAD),
           _rep(ln2_bs[0], HPAD),
           _rep(np.asarray(ln1_bs[1]) + tadds[1], HPAD),
           _rep(ln2_bs[1], HPAD)]

    in_maps = []
    for c in range(NCORES):
        xs = np.zeros((NPAD, INPAD), np.float32)
        xs[:NP, :IN_C] = x[c * NP:(c + 1) * NP]
        xT = xs.T.reshape(2, 128, NPAD).transpose(1, 0, 2).copy()
        m = {"xT": xT, "smat": smat[c], "gidx": gidx_w[c], "biasrow": biasrow}
        for i in range(6):
            m[f"w{i}"] = w_maps[i]
        for i in range(4):
            m[f"lng{i}"] = lng[i]
            m[f"lnb{i}"] = lnb[i]
        in_maps.append(m)
    return nchunk, in_maps


def kernel(**inputs):
    nchunk, in_maps = _prepare(inputs)
    if nchunk not in _CACHE:
        _CACHE[nchunk] = _build(nchunk)
    nc = _CACHE[nchunk]
    res = run_bass_kernel_spmd(nc, in_maps, list(range(NCORES)))
    out = np.empty((N, OUT_C), np.float32)
    for c in range(NCORES):
        o = res.results[c]["out"].reshape(NPAD, OPAD)
        out[c * NP:(c + 1) * NP] = o[:NP, :OUT_C]
    return out


# revision 10
# speedup vs baseline: 1264.0995x; 1264.0995x over previous
"""Trainium2 Bass kernel for nn_BasicGNN (4-layer GCN + 2 interleaved convs,
layer norms, time-embedding MLP) on N=10000 nodes / E=160000 edges.

Strategy: graph-parallel over 8 NeuronCores. Nodes are sharded contiguously
(1250/core). Per conv layer, each core:
  1. computes its h = x @ W shard on the PE (float32r),
  2. writes h (bf16, rows padded to 768) to HBM and AllGathers it,
  3. gathers the rows its local edges need with dma_gather and
     scatter-reduces them into dst rows via "staircase" matmuls whose
     stationary operand S carries the GCN edge coefficients (self loops and
     the conv bias ride along as extra slots),
  4. applies relu / layernorm / time-MLP epilogue on ACT+DVE.
Graph structure (edge partition, S matrices, gather indices) is precomputed
on the host from the actual edge_index; it is identical for all 6
message-passing rounds.
"""
import numpy as np
import ml_dtypes

import concourse.bass as bass
import concourse.tile as tile
from concourse import bacc, mybir
from concourse.bass_utils import run_bass_kernel_spmd

bf16 = ml_dtypes.bfloat16

# Problem shapes (hardcoded per spec nn_BasicGNN_41248865911519)
N, E = 10000, 160000
IN_C, HID, OUT_C, TDIM = 178, 712, 178, 32
EPS = 1e-5

NCORES = 8
NP = N // NCORES            # 1250 nodes per core
NT = 10                     # node tiles per core (9 full + 98)
NPAD = NT * 128             # 1280
HPAD = 768                  # padded hidden row (bf16 rows = 1536B, /256 ok)
INPAD = 256                 # padded input features (2 k-tiles)
OPAD = 256                  # padded output features (bf16 rows = 512B)
NBLK = 10                   # dst blocks of 128 per core
PIECE = 6                   # gather chunks per dma_gather call
BIAS_ROW = 1250             # local row in h_my carrying this layer's bias

F32, F32R, BF16, I16 = (mybir.dt.float32, mybir.dt.float32r,
                        mybir.dt.bfloat16, mybir.dt.int16)
AF = mybir.ActivationFunctionType
ALU = mybir.AluOpType


# ---------------------------------------------------------------- host prep
def _preprocess(edge_index):
    """Partition edges by dst core/block, dedup (block, src) into slots,
    build S matrices + gather index lists. Slot layout per (core, block):
    [unique srcs..., bias slot, padding]."""
    src = np.asarray(edge_index[0], np.int64)
    dst = np.asarray(edge_index[1], np.int64)
    deg = np.bincount(dst, minlength=N).astype(np.float32) + 1.0
    dinv = 1.0 / np.sqrt(deg)
    # append self loops
    a_src = np.concatenate([src, np.arange(N)])
    a_dst = np.concatenate([dst, np.arange(N)])
    a_coef = np.concatenate([dinv[src] * dinv[dst], dinv * dinv]).astype(np.float32)

    core = a_dst // NP
    loc = a_dst % NP
    blk = loc // 128
    row = loc % 128

    slots = [[None] * NBLK for _ in range(NCORES)]
    smats = [[None] * NBLK for _ in range(NCORES)]
    order = np.lexsort((a_src, blk, core))
    csrc, cblk, ccore = a_src[order], blk[order], core[order]
    crow, ccoef = row[order], a_coef[order]
    bounds = np.searchsorted(ccore * NBLK + cblk, np.arange(NCORES * NBLK + 1))
    max_nslot = 0
    for c in range(NCORES):
        for b in range(NBLK):
            g = slice(bounds[c * NBLK + b], bounds[c * NBLK + b + 1])
            gs, gr, gc = csrc[g], crow[g], ccoef[g]
            uniq, inv = np.unique(gs, return_inverse=True)
            S = np.zeros((len(uniq), 128), np.float32)
            np.add.at(S, (inv, gr), gc)
            slots[c][b] = uniq
            smats[c][b] = S
            max_nslot = max(max_nslot, len(uniq))
    nchunk = (max_nslot + 1 + 127) // 128   # +1 for the bias slot
    tot = NBLK * nchunk * 128

    def gid(n):  # padded global row id in the AllGathered buffer
        return (n // NP) * NPAD + (n % NP)

    gidx = np.zeros((NCORES, tot), np.int16)
    smat = np.zeros((NCORES, 128, NBLK * nchunk, 128), bf16)
    for c in range(NCORES):
        for b in range(NBLK):
            u = slots[c][b]
            S = smats[c][b]
            ns = len(u)
            base = b * nchunk * 128
            gidx[c, base:base + ns] = gid(u)
            for ch in range((ns + 127) // 128):
                lo, hi = ch * 128, min((ch + 1) * 128, ns)
                smat[c, 0:hi - lo, b * nchunk + ch, :] = S[lo:hi].astype(bf16)
            # bias slot: reads this core's h_my[BIAS_ROW], coef 1 everywhere
            gidx[c, base + ns] = c * NPAD + BIAS_ROW
            smat[c, ns % 128, b * nchunk + ns // 128, :] = bf16(1.0)
    # wrap: idx i at [i % 16, i // 16], replicated across the 8 Q7 cores
    gidx_w = gidx.reshape(NCORES, tot // 16, 16).transpose(0, 2, 1)
    gidx_w = np.tile(gidx_w, (1, 8, 1)).copy()
    return nchunk, gidx_w, smat


def _host_tmlp(timestep, time_w1, time_b1, time_ws, time_bs):
    t = np.asarray(timestep).astype(np.float32).reshape(-1)
    half = TDIM // 2
    freqs = np.exp(np.arange(half, dtype=np.float32)
                   * (-np.log(10000.0) / (half - 1)))
    args = t[:, None] * freqs[None, :]
    emb = np.concatenate([np.sin(args), np.cos(args)], -1).astype(np.float32)
    tm = np.maximum(emb @ np.asarray(time_w1, np.float32)
                    + np.asarray(time_b1, np.float32), 0.0)
    return [np.asarray(tm @ np.asarray(time_ws[j], np.float32)
                       + np.asarray(time_bs[j], np.float32)).reshape(-1)
            for j in range(2)]


def _padw(w, rpad, cpad):
    w = np.asarray(w, np.float32)
    out = np.zeros((rpad, cpad), np.float32)
    out[:w.shape[0], :w.shape[1]] = w
    return out


def _rep(v, cpad):
    v = np.asarray(v, np.float32).reshape(-1)
    out = np.zeros((128, cpad), np.float32)
    out[:, :v.shape[0]] = v[None, :]
    return out


# per-layer dense configs: (ktiles, outpad, out_valid)
LCFG = [(2, HPAD, HID), (6, HPAD, HID), (6, HPAD, HID),
        (6, HPAD, HID), (6, HPAD, HID), (6, OPAD, OUT_C)]
# post-ops per layer: (relu, ln param index or None)
POST = [(True, None), (True, 0), (True, 1), (True, 2), (True, 3),
        (False, None)]


# ------------------------------------------------------------- bass program
def _build(nchunk, reps=1):
    tot = NBLK * nchunk * 128
    nc = bacc.Bacc("TRN2", target_bir_lowering=False, debug=False,
                   num_devices=NCORES)

    xT_in = nc.dram_tensor("xT", [128, 2, NPAD], F32R, kind="ExternalInput")
    w_in = [nc.dram_tensor(f"w{i}", [128, kt, op], F32R, kind="ExternalInput")
            for i, (kt, op, _) in enumerate(LCFG)]
    biasrow_in = nc.dram_tensor("biasrow", [1, 6, HPAD], BF16,
                                kind="ExternalInput")
    lng_in = [nc.dram_tensor(f"lng{i}", [128, HPAD], F32, kind="ExternalInput")
              for i in range(4)]
    lnb_in = [nc.dram_tensor(f"lnb{i}", [128, HPAD], F32, kind="ExternalInput")
              for i in range(4)]
    s_in = nc.dram_tensor("smat", [128, NBLK * nchunk, 128], BF16,
                          kind="ExternalInput")
    gidx_in = nc.dram_tensor("gidx", [128, tot // 16], I16,
                             kind="ExternalInput")

    out_dram = nc.dram_tensor("out", [NT, 128, OPAD], F32,
                              kind="ExternalOutput")

    h_my = nc.dram_tensor("h_my", [NPAD, HPAD], BF16)
    h_full = nc.dram_tensor("h_full", [NCORES * NPAD, HPAD], BF16,
                            addr_space="Shared")
    h_my_sm = nc.dram_tensor("h_my_sm", [NPAD, OPAD], BF16)
    h_full_sm = nc.dram_tensor("h_full_sm", [NCORES * NPAD, OPAD], BF16,
                               addr_space="Shared")

    with tile.TileContext(nc) as tc:
        import contextlib
        with contextlib.ExitStack() as ctx:
            env = _setup(ctx, tc, nc, nchunk, xT_in, lng_in, lnb_in,
                         s_in, gidx_in, biasrow_in)
            for _ in range(reps):
                _layers(tc, nc, nchunk, env, w_in, out_dram, h_my, h_full,
                        h_my_sm, h_full_sm)
    nc.compile()
    return nc


def _setup(ctx, tc, nc, nchunk, xT_in, lng_in, lnb_in, s_in, gidx_in,
           biasrow_in):
    tot = NBLK * nchunk * 128

    const = ctx.enter_context(tc.tile_pool(name="const", bufs=1))
    wpool = ctx.enter_context(tc.tile_pool(name="wpool", bufs=1))
    act_pool = ctx.enter_context(tc.tile_pool(name="act", bufs=1))
    xt_pool = ctx.enter_context(tc.tile_pool(name="xt", bufs=4))
    hbf_pool = ctx.enter_context(tc.tile_pool(name="hbf", bufs=3))
    gath_pool = ctx.enter_context(tc.tile_pool(name="gath", bufs=3))
    ep_pool = ctx.enter_context(tc.tile_pool(name="ep", bufs=2))
    st_pool = ctx.enter_context(tc.tile_pool(name="st", bufs=4))
    psum_h = ctx.enter_context(tc.tile_pool(name="ph", bufs=1, space="PSUM"))
    psum_t = ctx.enter_context(tc.tile_pool(name="pt", bufs=2, space="PSUM"))
    psum_a = ctx.enter_context(tc.tile_pool(name="pa", bufs=2, space="PSUM"))

    # ---- persistent loads
    smat = const.tile([128, NBLK * nchunk, 128], BF16)
    nc.sync.dma_start(smat[:], s_in[:, :, :])
    gidx = const.tile([128, tot // 16], I16)
    nc.sync.dma_start(gidx[:], gidx_in[:, :])
    biasrow = const.tile([1, 6, HPAD], BF16)
    nc.sync.dma_start(biasrow[:], biasrow_in[:, :, :])
    lngs, lnbs = [], []
    for i in range(4):
        g = const.tile([128, HPAD], F32, tag=f"lng{i}")
        nc.sync.dma_start(g[:], lng_in[i][:, :])
        lngs.append(g)
        b = const.tile([128, HPAD], F32, tag=f"lnb{i}")
        nc.sync.dma_start(b[:], lnb_in[i][:, :])
        lnbs.append(b)
    ident = const.tile([128, 128], F32)
    nc.sync.dma_start(ident[:], nc.inline_tensor(np.eye(128, dtype=np.float32),
                                                 "ident").ap())
    xT0 = const.tile([128, 2, NPAD], F32R)
    nc.sync.dma_start(xT0[:], xT_in[:, :, :])

    # activation (node-major), rewritten by each layer's epilogue
    act = act_pool.tile([128, NT, HPAD], F32)
    return dict(smat=smat, gidx=gidx, biasrow=biasrow, lngs=lngs, lnbs=lnbs,
                ident=ident, xT0=xT0, act=act, wpool=wpool, xt_pool=xt_pool,
                hbf_pool=hbf_pool, gath_pool=gath_pool, ep_pool=ep_pool,
                st_pool=st_pool, psum_h=psum_h, psum_t=psum_t, psum_a=psum_a)


def _layers(tc, nc, nchunk, env, w_in, out_dram, h_my, h_full,
            h_my_sm, h_full_sm):
    smat, gidx, biasrow = env["smat"], env["gidx"], env["biasrow"]
    lngs, lnbs, ident, xT0, act = (env["lngs"], env["lnbs"], env["ident"],
                                   env["xT0"], env["act"])
    wpool, xt_pool, hbf_pool = env["wpool"], env["xt_pool"], env["hbf_pool"]
    gath_pool, ep_pool, st_pool = env["gath_pool"], env["ep_pool"], env["st_pool"]
    psum_h, psum_t, psum_a = env["psum_h"], env["psum_t"], env["psum_a"]

    for L, (kt_n, op, ov) in enumerate(LCFG):
        hm = h_my if L < 5 else h_my_sm
        hf = h_full if L < 5 else h_full_sm
        # ---------- phase A: dense h = x @ W -> bf16 -> HBM
        w_r = wpool.tile([128, kt_n, op], F32R, tag="w_r")
        nc.sync.dma_start(w_r[:], w_in[L][:, :, :])
        for nt in range(NT):
            hp = psum_h.tile([128, op], F32, tag="hpsum")
            for kt in range(kt_n):
                if L == 0:
                    xt = xT0[:, kt, nt * 128:(nt + 1) * 128]
                else:
                    tp = psum_t.tile([128, 128], F32, tag="tp")
                    nc.tensor.transpose(
                        tp[:], act[:, nt, kt * 128:(kt + 1) * 128], ident[:])
                    xts = xt_pool.tile([128, 128], F32R, tag="xt")
                    nc.scalar.copy(xts[:], tp[:])
                    xt = xts[:]
                for lo in range(0, op, 512):
                    hi = min(lo + 512, op)
                    nc.tensor.matmul(hp[:, lo:hi], xt, w_r[:, kt, lo:hi],
                                     start=(kt == 0), stop=(kt == kt_n - 1))
            hbf = hbf_pool.tile([128, op], BF16, tag="hbf")
            nc.vector.tensor_copy(hbf[:], hp[:])
            nc.sync.dma_start(hm[nt * 128:(nt + 1) * 128, :], hbf[:])
        # bias row for the aggregation's bias slot
        nc.sync.dma_start(hm[BIAS_ROW:BIAS_ROW + 1, :op],
                          biasrow[0:1, L, 0:op])
        # ---------- phase B: allgather
        nc.gpsimd.collective_compute(
            "AllGather", ALU.bypass,
            replica_groups=[list(range(NCORES))],
            ins=[hm.ap().opt()], outs=[hf.ap().opt()])
        # ---------- phase C: gather + aggregate + epilogue per dst block
        relu, ln = POST[L]
        for b in range(NBLK):
            ap_ = psum_a.tile([128, op], F32, tag="apsum")
            npieces = (nchunk + PIECE - 1) // PIECE
            for p in range(npieces):
                c0, c1 = p * PIECE, min((p + 1) * PIECE, nchunk)
                g = gath_pool.tile([128, c1 - c0, op], BF16, tag="gath")
                i0 = (b * nchunk + c0) * 128
                i1 = (b * nchunk + c1) * 128
                nc.gpsimd.dma_gather(
                    g[:], hf.ap(), gidx[:, i0 // 16:i1 // 16],
                    i1 - i0, i1 - i0, op)
                for c in range(c0, c1):
                    for lo in range(0, op, 512):
                        hi = min(lo + 512, op)
                        nc.tensor.matmul(
                            ap_[:, lo:hi], smat[:, b * nchunk + c, :],
                            g[:, c - c0, lo:hi],
                            start=(c == 0), stop=(c == nchunk - 1))
            _epilogue(nc, ep_pool, st_pool, ap_, act, out_dram, b, relu, ln,
                      lngs, lnbs, op, ov, L)


def _epilogue(nc, ep_pool, st_pool, ap_, act, out_dram, b, relu, ln,
              lngs, lnbs, op, ov, L):
    """psum [128, op] (already includes bias) -> relu -> LN*g+b -> dst"""
    if L == 5:
        t = ep_pool.tile([128, op], F32, tag="ep5")
        nc.vector.tensor_copy(t[:], ap_[:])
        nc.sync.dma_start(out_dram[b, :, :], t[:])
        return
    dstv = act[:, b, :]
    if ln is None:
        nc.scalar.activation(dstv, ap_[:], AF.Relu)
        return
    # relu (psum -> sbuf) with fused sum; then square with fused sum
    t = ep_pool.tile([128, op], F32, tag="ep")
    stats = st_pool.tile([128, 2], F32, tag="st")
    nc.scalar.activation(t[:], ap_[:], AF.Relu, accum_out=stats[:, 0:1])
    x = t[:, 0:ov]
    sq = ep_pool.tile([128, ov], F32, tag="scr")
    nc.scalar.activation(sq[:], x, AF.Square, accum_out=stats[:, 1:2])
    nmu = st_pool.tile([128, 1], F32, tag="nmu")
    nc.scalar.mul(nmu[:], stats[:, 0:1], -1.0 / ov)
    mu2 = st_pool.tile([128, 1], F32, tag="mu2")
    nc.vector.tensor_mul(mu2[:], nmu[:], nmu[:])
    var = st_pool.tile([128, 1], F32, tag="var")
    nc.vector.tensor_scalar(var[:], stats[:, 1:2], 1.0 / ov, None,
                            op0=ALU.mult)
    nc.vector.tensor_sub(var[:], var[:], mu2[:])
    nc.vector.tensor_scalar_add(var[:], var[:], EPS)
    rvar = st_pool.tile([128, 1], F32, tag="rvar")
    nc.vector.reciprocal(rvar[:], var[:])
    rstd = st_pool.tile([128, 1], F32, tag="rstd")
    nc.scalar.activation(rstd[:], rvar[:], AF.Sqrt)
    y = ep_pool.tile([128, ov], F32, tag="scr")
    nc.vector.tensor_scalar(y[:], x, nmu[:], rstd[:],
                            op0=ALU.add, op1=ALU.mult)
    g, bb = lngs[ln], lnbs[ln]
    nc.vector.tensor_mul(y[:], y[:], g[:, 0:ov])
    nc.vector.tensor_add(act[:, b, 0:ov], y[:], bb[:, 0:ov])


# ------------------------------------------------------------------- driver
_CACHE = {}


def _prepare(inputs):
    x = np.asarray(inputs["x"], np.float32)
    edge_index = np.asarray(inputs["edge_index"])
    nchunk, gidx_w, smat = _preprocess(edge_index)

    tadds = _host_tmlp(inputs["timestep"], inputs["time_w1"],
                       inputs["time_b1"], inputs["time_ws"], inputs["time_bs"])

    conv_ws = [np.asarray(w, np.float32) for w in inputs["conv_ws"]]
    conv_bs = [np.asarray(w, np.float32) for w in inputs["conv_bs"]]
    conv2_ws = [np.asarray(w, np.float32) for w in inputs["conv2_ws"]]
    conv2_bs = [np.asarray(w, np.float32) for w in inputs["conv2_bs"]]
    ln1_gs = [np.asarray(w, np.float32) for w in inputs["ln1_gs"]]
    ln1_bs = [np.asarray(w, np.float32) for w in inputs["ln1_bs"]]
    ln2_gs = [np.asarray(w, np.float32) for w in inputs["ln2_gs"]]
    ln2_bs = [np.asarray(w, np.float32) for w in inputs["ln2_bs"]]

    # layer order: conv0, conv1, conv2_0, conv2, conv2_1, conv3
    Ws = [conv_ws[0], conv_ws[1], conv2_ws[0], conv_ws[2], conv2_ws[1],
          conv_ws[3]]
    Bs = [conv_bs[0], conv_bs[1], conv2_bs[0], conv_bs[2], conv2_bs[1],
          conv_bs[3]]
    w_maps = []
    biasrow = np.zeros((1, 6, HPAD), bf16)
    for i, (kt, op, _) in enumerate(LCFG):
        rp = INPAD if i == 0 else HPAD
        wp = _padw(Ws[i], rp, op)
        w_maps.append(wp.reshape(kt, 128, op).transpose(1, 0, 2).copy())
        biasrow[0, i, :op] = _padw(Bs[i].reshape(1, -1), 1, op)[0].astype(bf16)
    lng = [_rep(ln1_gs[0], HPAD), _rep(ln2_gs[0], HPAD),
           _rep(ln1_gs[1], HPAD), _rep(ln2_gs[1], HPAD)]
    lnb = [_rep(np.asarray(ln1_bs[0]) + tadds[0], HPAD),
           _rep(ln2_bs[0], HPAD),
           _rep(np.asarray(ln1_bs[1]) + tadds[1], HPAD),
           _rep(ln2_bs[1], HPAD)]

    in_maps = []
    for c in range(NCORES):
        xs = np.zeros((NPAD, INPAD), np.float32)
        xs[:NP, :IN_C] = x[c * NP:(c + 1) * NP]
        xT = xs.T.reshape(2, 128, NPAD).transpose(1, 0, 2).copy()
        m = {"xT": xT, "smat": smat[c], "gidx": gidx_w[c], "biasrow": biasrow}
        for i in range(6):
            m[f"w{i}"] = w_maps[i]
        for i in range(4):
            m[f"lng{i}"] = lng[i]
            m[f"lnb{i}"] = lnb[i]
        in_maps.append(m)
    return nchunk, in_maps


def kernel(**inputs):
    nchunk, in_maps = _prepare(inputs)
    if nchunk not in _CACHE:
        _CACHE[nchunk] = _build(nchunk)
    nc = _CACHE[nchunk]
    res = run_bass_kernel_spmd(nc, in_maps, list(range(NCORES)))
    out = np.empty((N, OUT_C), np.float32)
    for c in range(NCORES):
        o = res.results[c]["out"].reshape(NPAD, OPAD)
        out[c * NP:(c + 1) * NP] = o[:NP, :OUT_C]
    return out


# revision 11
# speedup vs baseline: 2157.8550x; 1.7070x over previous
"""Trainium2 Bass kernel for nn_BasicGNN (4-layer GCN + 2 interleaved convs,
layer norms, time-embedding MLP) on N=10000 nodes / E=160000 edges.

Strategy: graph-parallel over 8 NeuronCores. Nodes are sharded contiguously
(1250/core). Per conv layer, each core:
  1. computes its h = x @ W shard on the PE (float32r),
  2. writes h (bf16, rows padded to 768) to HBM and AllGathers it,
  3. gathers the rows its local edges need with dma_gather and
     scatter-reduces them into dst rows via "staircase" matmuls whose
     stationary operand S carries the GCN edge coefficients (self loops and
     the conv bias ride along as extra slots),
  4. applies relu / layernorm / time-MLP epilogue on ACT+DVE.
Graph structure (edge partition, S matrices, gather indices) is precomputed
on the host from the actual edge_index; it is identical for all 6
message-passing rounds.
"""
import numpy as np
import ml_dtypes

import concourse.bass as bass
import concourse.tile as tile
from concourse import bacc, mybir
from concourse.bass_utils import run_bass_kernel_spmd

bf16 = ml_dtypes.bfloat16

# Problem shapes (hardcoded per spec nn_BasicGNN_41248865911519)
N, E = 10000, 160000
IN_C, HID, OUT_C, TDIM = 178, 712, 178, 32
EPS = 1e-5

NCORES = 8
NP = N // NCORES            # 1250 nodes per core
NT = 10                     # node tiles per core (9 full + 98)
NPAD = NT * 128             # 1280
HPAD = 768                  # padded hidden row (bf16 rows = 1536B, /256 ok)
INPAD = 256                 # padded input features (2 k-tiles)
OPAD = 256                  # padded output features (bf16 rows = 512B)
NBLK = 10                   # dst blocks of 128 per core
PIECE = 6                   # gather chunks per dma_gather call
BIAS_ROW = 1250             # local row in h_my carrying this layer's bias

F32, F32R, BF16, I16 = (mybir.dt.float32, mybir.dt.float32r,
                        mybir.dt.bfloat16, mybir.dt.int16)
AF = mybir.ActivationFunctionType
ALU = mybir.AluOpType


# ---------------------------------------------------------------- host prep
def _preprocess(edge_index):
    """Partition edges by dst core/block, dedup (block, src) into slots,
    build S matrices + gather index lists. Slot layout per (core, block):
    [unique srcs..., bias slot, padding]."""
    src = np.asarray(edge_index[0], np.int64)
    dst = np.asarray(edge_index[1], np.int64)
    deg = np.bincount(dst, minlength=N).astype(np.float32) + 1.0
    dinv = 1.0 / np.sqrt(deg)
    # append self loops
    a_src = np.concatenate([src, np.arange(N)])
    a_dst = np.concatenate([dst, np.arange(N)])
    a_coef = np.concatenate([dinv[src] * dinv[dst], dinv * dinv]).astype(np.float32)

    core = a_dst // NP
    loc = a_dst % NP
    blk = loc // 128
    row = loc % 128

    slots = [[None] * NBLK for _ in range(NCORES)]
    smats = [[None] * NBLK for _ in range(NCORES)]
    order = np.lexsort((a_src, blk, core))
    csrc, cblk, ccore = a_src[order], blk[order], core[order]
    crow, ccoef = row[order], a_coef[order]
    bounds = np.searchsorted(ccore * NBLK + cblk, np.arange(NCORES * NBLK + 1))
    max_nslot = 0
    for c in range(NCORES):
        for b in range(NBLK):
            g = slice(bounds[c * NBLK + b], bounds[c * NBLK + b + 1])
            gs, gr, gc = csrc[g], crow[g], ccoef[g]
            uniq, inv = np.unique(gs, return_inverse=True)
            S = np.zeros((len(uniq), 128), np.float32)
            np.add.at(S, (inv, gr), gc)
            slots[c][b] = uniq
            smats[c][b] = S
            max_nslot = max(max_nslot, len(uniq))
    nchunk = (max_nslot + 1 + 127) // 128   # +1 for the bias slot
    tot = NBLK * nchunk * 128

    def gid(n):  # padded global row id in the AllGathered buffer
        return (n // NP) * NPAD + (n % NP)

    gidx = np.zeros((NCORES, tot), np.int16)
    smat = np.zeros((NCORES, 128, NBLK * nchunk, 128), bf16)
    for c in range(NCORES):
        for b in range(NBLK):
            u = slots[c][b]
            S = smats[c][b]
            ns = len(u)
            base = b * nchunk * 128
            gidx[c, base:base + ns] = gid(u)
            for ch in range((ns + 127) // 128):
                lo, hi = ch * 128, min((ch + 1) * 128, ns)
                smat[c, 0:hi - lo, b * nchunk + ch, :] = S[lo:hi].astype(bf16)
            # bias slot: reads this core's h_my[BIAS_ROW], coef 1 everywhere
            gidx[c, base + ns] = c * NPAD + BIAS_ROW
            smat[c, ns % 128, b * nchunk + ns // 128, :] = bf16(1.0)
    # wrap: idx i at [i % 16, i // 16], replicated across the 8 Q7 cores
    gidx_w = gidx.reshape(NCORES, tot // 16, 16).transpose(0, 2, 1)
    gidx_w = np.tile(gidx_w, (1, 8, 1)).copy()
    return nchunk, gidx_w, smat


def _host_tmlp(timestep, time_w1, time_b1, time_ws, time_bs):
    t = np.asarray(timestep).astype(np.float32).reshape(-1)
    half = TDIM // 2
    freqs = np.exp(np.arange(half, dtype=np.float32)
                   * (-np.log(10000.0) / (half - 1)))
    args = t[:, None] * freqs[None, :]
    emb = np.concatenate([np.sin(args), np.cos(args)], -1).astype(np.float32)
    tm = np.maximum(emb @ np.asarray(time_w1, np.float32)
                    + np.asarray(time_b1, np.float32), 0.0)
    return [np.asarray(tm @ np.asarray(time_ws[j], np.float32)
                       + np.asarray(time_bs[j], np.float32)).reshape(-1)
            for j in range(2)]


def _padw(w, rpad, cpad):
    w = np.asarray(w, np.float32)
    out = np.zeros((rpad, cpad), np.float32)
    out[:w.shape[0], :w.shape[1]] = w
    return out


def _rep(v, cpad):
    v = np.asarray(v, np.float32).reshape(-1)
    out = np.zeros((128, cpad), np.float32)
    out[:, :v.shape[0]] = v[None, :]
    return out


# per-layer dense configs: (ktiles, outpad, out_valid)
LCFG = [(2, HPAD, HID), (6, HPAD, HID), (6, HPAD, HID),
        (6, HPAD, HID), (6, HPAD, HID), (6, OPAD, OUT_C)]
# post-ops per layer: (relu, ln param index or None)
POST = [(True, None), (True, 0), (True, 1), (True, 2), (True, 3),
        (False, None)]


# ------------------------------------------------------------- bass program
def _build(nchunk, reps=1, single=False):
    tot = NBLK * nchunk * 128
    nc = bacc.Bacc("TRN2", target_bir_lowering=False, debug=False,
                   num_devices=1 if single else NCORES)

    xT_in = nc.dram_tensor("xT", [128, 2, NPAD], F32R, kind="ExternalInput")
    w_in = [nc.dram_tensor(f"w{i}", [128, kt, op], F32R, kind="ExternalInput")
            for i, (kt, op, _) in enumerate(LCFG)]
    biasrow_in = nc.dram_tensor("biasrow", [1, 6, HPAD], BF16,
                                kind="ExternalInput")
    lng_in = [nc.dram_tensor(f"lng{i}", [128, HPAD], F32, kind="ExternalInput")
              for i in range(4)]
    lnb_in = [nc.dram_tensor(f"lnb{i}", [128, HPAD], F32, kind="ExternalInput")
              for i in range(4)]
    s_in = nc.dram_tensor("smat", [128, NBLK * nchunk, 128], BF16,
                          kind="ExternalInput")
    gidx_in = nc.dram_tensor("gidx", [128, tot // 16], I16,
                             kind="ExternalInput")

    out_dram = nc.dram_tensor("out", [NT, 128, OPAD], F32,
                              kind="ExternalOutput")

    h_my = nc.dram_tensor("h_my", [NPAD, HPAD], BF16)
    h_full = nc.dram_tensor("h_full", [NCORES * NPAD, HPAD], BF16,
                            addr_space="Shared")
    h_my_sm = nc.dram_tensor("h_my_sm", [NPAD, OPAD], BF16)
    h_full_sm = nc.dram_tensor("h_full_sm", [NCORES * NPAD, OPAD], BF16,
                               addr_space="Shared")

    with tile.TileContext(nc) as tc:
        import contextlib
        with contextlib.ExitStack() as ctx:
            env = _setup(ctx, tc, nc, nchunk, xT_in, lng_in, lnb_in,
                         s_in, gidx_in, biasrow_in)
            for _ in range(reps):
                _layers(tc, nc, nchunk, env, w_in, out_dram, h_my, h_full,
                        h_my_sm, h_full_sm, single)
    nc.compile()
    return nc


def _setup(ctx, tc, nc, nchunk, xT_in, lng_in, lnb_in, s_in, gidx_in,
           biasrow_in):
    tot = NBLK * nchunk * 128

    const = ctx.enter_context(tc.tile_pool(name="const", bufs=1))
    wpool = ctx.enter_context(tc.tile_pool(name="wpool", bufs=1))
    act_pool = ctx.enter_context(tc.tile_pool(name="act", bufs=1))
    xt_pool = ctx.enter_context(tc.tile_pool(name="xt", bufs=4))
    hbf_pool = ctx.enter_context(tc.tile_pool(name="hbf", bufs=3))
    gath_pool = ctx.enter_context(tc.tile_pool(name="gath", bufs=3))
    ep_pool = ctx.enter_context(tc.tile_pool(name="ep", bufs=2))
    st_pool = ctx.enter_context(tc.tile_pool(name="st", bufs=4))
    psum_h = ctx.enter_context(tc.tile_pool(name="ph", bufs=1, space="PSUM"))
    psum_t = ctx.enter_context(tc.tile_pool(name="pt", bufs=2, space="PSUM"))
    psum_a = ctx.enter_context(tc.tile_pool(name="pa", bufs=2, space="PSUM"))

    # ---- persistent loads
    smat = const.tile([128, NBLK * nchunk, 128], BF16)
    nc.sync.dma_start(smat[:], s_in[:, :, :])
    gidx = const.tile([128, tot // 16], I16)
    nc.sync.dma_start(gidx[:], gidx_in[:, :])
    biasrow = const.tile([1, 6, HPAD], BF16)
    nc.sync.dma_start(biasrow[:], biasrow_in[:, :, :])
    lngs, lnbs = [], []
    for i in range(4):
        g = const.tile([128, HPAD], F32, tag=f"lng{i}")
        nc.sync.dma_start(g[:], lng_in[i][:, :])
        lngs.append(g)
        b = const.tile([128, HPAD], F32, tag=f"lnb{i}")
        nc.sync.dma_start(b[:], lnb_in[i][:, :])
        lnbs.append(b)
    ident = const.tile([128, 128], F32)
    nc.sync.dma_start(ident[:], nc.inline_tensor(np.eye(128, dtype=np.float32),
                                                 "ident").ap())
    xT0 = const.tile([128, 2, NPAD], F32R)
    nc.sync.dma_start(xT0[:], xT_in[:, :, :])

    # activation (node-major), rewritten by each layer's epilogue
    act = act_pool.tile([128, NT, HPAD], F32)
    return dict(smat=smat, gidx=gidx, biasrow=biasrow, lngs=lngs, lnbs=lnbs,
                ident=ident, xT0=xT0, act=act, wpool=wpool, xt_pool=xt_pool,
                hbf_pool=hbf_pool, gath_pool=gath_pool, ep_pool=ep_pool,
                st_pool=st_pool, psum_h=psum_h, psum_t=psum_t, psum_a=psum_a)


def _layers(tc, nc, nchunk, env, w_in, out_dram, h_my, h_full,
            h_my_sm, h_full_sm, single=False):
    smat, gidx, biasrow = env["smat"], env["gidx"], env["biasrow"]
    lngs, lnbs, ident, xT0, act = (env["lngs"], env["lnbs"], env["ident"],
                                   env["xT0"], env["act"])
    wpool, xt_pool, hbf_pool = env["wpool"], env["xt_pool"], env["hbf_pool"]
    gath_pool, ep_pool, st_pool = env["gath_pool"], env["ep_pool"], env["st_pool"]
    psum_h, psum_t, psum_a = env["psum_h"], env["psum_t"], env["psum_a"]

    for L, (kt_n, op, ov) in enumerate(LCFG):
        hm = h_my if L < 5 else h_my_sm
        hf = h_full if L < 5 else h_full_sm
        # ---------- phase A: dense h = x @ W -> bf16 -> HBM
        w_r = wpool.tile([128, kt_n, op], F32R, tag="w_r")
        nc.sync.dma_start(w_r[:], w_in[L][:, :, :])
        for nt in range(NT):
            hp = psum_h.tile([128, op], F32, tag="hpsum")
            for kt in range(kt_n):
                if L == 0:
                    xt = xT0[:, kt, nt * 128:(nt + 1) * 128]
                else:
                    tp = psum_t.tile([128, 128], F32, tag="tp")
                    nc.tensor.transpose(
                        tp[:], act[:, nt, kt * 128:(kt + 1) * 128], ident[:])
                    xts = xt_pool.tile([128, 128], F32R, tag="xt")
                    nc.scalar.copy(xts[:], tp[:])
                    xt = xts[:]
                for lo in range(0, op, 512):
                    hi = min(lo + 512, op)
                    nc.tensor.matmul(hp[:, lo:hi], xt, w_r[:, kt, lo:hi],
                                     start=(kt == 0), stop=(kt == kt_n - 1))
            hbf = hbf_pool.tile([128, op], BF16, tag="hbf")
            nc.vector.tensor_copy(hbf[:], hp[:])
            nc.sync.dma_start(hm[nt * 128:(nt + 1) * 128, :], hbf[:])
        # bias row for the aggregation's bias slot
        nc.sync.dma_start(hm[BIAS_ROW:BIAS_ROW + 1, :op],
                          biasrow[0:1, L, 0:op])
        # ---------- phase B: allgather
        if not single:
            nc.gpsimd.collective_compute(
                "AllGather", ALU.bypass,
                replica_groups=[list(range(NCORES))],
                ins=[hm.ap().opt()], outs=[hf.ap().opt()])
        # ---------- phase C: gather + aggregate + epilogue per dst block
        relu, ln = POST[L]
        for b in range(NBLK):
            ap_ = psum_a.tile([128, op], F32, tag="apsum")
            npieces = (nchunk + PIECE - 1) // PIECE
            for p in range(npieces):
                c0, c1 = p * PIECE, min((p + 1) * PIECE, nchunk)
                g = gath_pool.tile([128, c1 - c0, op], BF16, tag="gath")
                i0 = (b * nchunk + c0) * 128
                i1 = (b * nchunk + c1) * 128
                nc.gpsimd.dma_gather(
                    g[:], hf.ap(), gidx[:, i0 // 16:i1 // 16],
                    i1 - i0, i1 - i0, op)
                for c in range(c0, c1):
                    for lo in range(0, op, 512):
                        hi = min(lo + 512, op)
                        nc.tensor.matmul(
                            ap_[:, lo:hi], smat[:, b * nchunk + c, :],
                            g[:, c - c0, lo:hi],
                            start=(c == 0), stop=(c == nchunk - 1))
            _epilogue(nc, ep_pool, st_pool, ap_, act, out_dram, b, relu, ln,
                      lngs, lnbs, op, ov, L)


def _epilogue(nc, ep_pool, st_pool, ap_, act, out_dram, b, relu, ln,
              lngs, lnbs, op, ov, L):
    """psum [128, op] (already includes bias) -> relu -> LN*g+b -> dst"""
    if L == 5:
        t = ep_pool.tile([128, op], F32, tag="ep5")
        nc.vector.tensor_copy(t[:], ap_[:])
        nc.sync.dma_start(out_dram[b, :, :], t[:])
        return
    dstv = act[:, b, :]
    if ln is None:
        nc.scalar.activation(dstv, ap_[:], AF.Relu)
        return
    # relu (psum -> sbuf) with fused sum; then square with fused sum
    t = ep_pool.tile([128, op], F32, tag="ep")
    stats = st_pool.tile([128, 2], F32, tag="st")
    nc.scalar.activation(t[:], ap_[:], AF.Relu, accum_out=stats[:, 0:1])
    x = t[:, 0:ov]
    sq = ep_pool.tile([128, ov], F32, tag="scr")
    nc.scalar.activation(sq[:], x, AF.Square, accum_out=stats[:, 1:2])
    nmu = st_pool.tile([128, 1], F32, tag="nmu")
    nc.scalar.mul(nmu[:], stats[:, 0:1], -1.0 / ov)
    mu2 = st_pool.tile([128, 1], F32, tag="mu2")
    nc.vector.tensor_mul(mu2[:], nmu[:], nmu[:])
    var = st_pool.tile([128, 1], F32, tag="var")
    nc.vector.tensor_scalar(var[:], stats[:, 1:2], 1.0 / ov, None,
                            op0=ALU.mult)
    nc.vector.tensor_sub(var[:], var[:], mu2[:])
    nc.vector.tensor_scalar_add(var[:], var[:], EPS)
    rvar = st_pool.tile([128, 1], F32, tag="rvar")
    nc.vector.reciprocal(rvar[:], var[:])
    rstd = st_pool.tile([128, 1], F32, tag="rstd")
    nc.scalar.activation(rstd[:], rvar[:], AF.Sqrt)
    y = ep_pool.tile([128, ov], F32, tag="scr")
    nc.vector.tensor_scalar(y[:], x, nmu[:], rstd[:],
                            op0=ALU.add, op1=ALU.mult)
    g, bb = lngs[ln], lnbs[ln]
    nc.vector.tensor_mul(y[:], y[:], g[:, 0:ov])
    nc.vector.tensor_add(act[:, b, 0:ov], y[:], bb[:, 0:ov])


# ------------------------------------------------------------------- driver
_CACHE = {}


def _prepare(inputs):
    x = np.asarray(inputs["x"], np.float32)
    edge_index = np.asarray(inputs["edge_index"])
    nchunk, gidx_w, smat = _preprocess(edge_index)

    tadds = _host_tmlp(inputs["timestep"], inputs["time_w1"],
                       inputs["time_b1"], inputs["time_ws"], inputs["time_bs"])

    conv_ws = [np.asarray(w, np.float32) for w in inputs["conv_ws"]]
    conv_bs = [np.asarray(w, np.float32) for w in inputs["conv_bs"]]
    conv2_ws = [np.asarray(w, np.float32) for w in inputs["conv2_ws"]]
    conv2_bs = [np.asarray(w, np.float32) for w in inputs["conv2_bs"]]
    ln1_gs = [np.asarray(w, np.float32) for w in inputs["ln1_gs"]]
    ln1_bs = [np.asarray(w, np.float32) for w in inputs["ln1_bs"]]
    ln2_gs = [np.asarray(w, np.float32) for w in inputs["ln2_gs"]]
    ln2_bs = [np.asarray(w, np.float32) for w in inputs["ln2_bs"]]

    # layer order: conv0, conv1, conv2_0, conv2, conv2_1, conv3
    Ws = [conv_ws[0], conv_ws[1], conv2_ws[0], conv_ws[2], conv2_ws[1],
          conv_ws[3]]
    Bs = [conv_bs[0], conv_bs[1], conv2_bs[0], conv_bs[2], conv2_bs[1],
          conv_bs[3]]
    w_maps = []
    biasrow = np.zeros((1, 6, HPAD), bf16)
    for i, (kt, op, _) in enumerate(LCFG):
        rp = INPAD if i == 0 else HPAD
        wp = _padw(Ws[i], rp, op)
        w_maps.append(wp.reshape(kt, 128, op).transpose(1, 0, 2).copy())
        biasrow[0, i, :op] = _padw(Bs[i].reshape(1, -1), 1, op)[0].astype(bf16)
    lng = [_rep(ln1_gs[0], HPAD), _rep(ln2_gs[0], HPAD),
           _rep(ln1_gs[1], HPAD), _rep(ln2_gs[1], HPAD)]
    lnb = [_rep(np.asarray(ln1_bs[0]) + tadds[0], HPAD),
           _rep(ln2_bs[0], HPAD),
           _rep(np.asarray(ln1_bs[1]) + tadds[1], HPAD),
           _rep(ln2_bs[1], HPAD)]

    in_maps = []
    for c in range(NCORES):
        xs = np.zeros((NPAD, INPAD), np.float32)
        xs[:NP, :IN_C] = x[c * NP:(c + 1) * NP]
        xT = xs.T.reshape(2, 128, NPAD).transpose(1, 0, 2).copy()
        m = {"xT": xT, "smat": smat[c], "gidx": gidx_w[c], "biasrow": biasrow}
        for i in range(6):
            m[f"w{i}"] = w_maps[i]
        for i in range(4):
            m[f"lng{i}"] = lng[i]
            m[f"lnb{i}"] = lnb[i]
        in_maps.append(m)
    return nchunk, in_maps


def kernel(**inputs):
    nchunk, in_maps = _prepare(inputs)
    if nchunk not in _CACHE:
        _CACHE[nchunk] = _build(nchunk)
    nc = _CACHE[nchunk]
    res = run_bass_kernel_spmd(nc, in_maps, list(range(NCORES)))
    out = np.empty((N, OUT_C), np.float32)
    for c in range(NCORES):
        o = res.results[c]["out"].reshape(NPAD, OPAD)
        out[c * NP:(c + 1) * NP] = o[:NP, :OUT_C]
    return out
